# revision 39
# baseline (speedup 1.0000x reference)
"""Multi-head self-attention (B=2, S=2048, E=1024, H=16, D=64) on 8 TRN2 cores.

Sharding: tensor-parallel over (batch, head-group): core c handles batch c//4
and heads [4*(c%4), 4*(c%4)+4). Each core computes its heads' attention output
projected through its slice of Wo; the host sums the 4 partial outputs per
batch and adds the constant bias row (bv @ Wo + bo).

Device-side math (per core, transposed formulation so no transposes needed):
  QT = Wq_c^T @ x^T + bq_c        [256, S]   (bias bk dropped: softmax-invariant)
  KT = Wk_c^T @ x^T               [256, S]
  V  = x @ Wv_c                   [S, 256]   (bias bv folded into host bias row)
  S^T tile = K @ Q^T              (PE, per 128-k-token x 1024-q tile)
  P^T = exp(S^T / 8)              (ACT, no max subtraction: scores ~ N(0,1))
  O^T aug = [V | 1]^T @ P^T       (PE, accumulated over k tiles; row 64 = sum)
  O^T = O^T aug[0:64] / row 64    (recip + PE broadcast + DVE mul)
  Y = O @ Wo_c                    [S, 1024]  fp32 partial out
"""

import numpy as np
import ml_dtypes

import concourse.bass as bass
import concourse.bacc as bacc
import concourse.tile as tile
from concourse import mybir
from concourse.bass_utils import run_bass_kernel_spmd

B, S, E = 2, 2048, 1024
H, D = 16, 64
NCORES = 8
HPC = 4                 # heads per core
EH = HPC * D            # 256: per-core head width
P = 128
EC = E // P             # 8 E-chunks of 128
MC = EH // P            # 2 Eh-chunks of 128
NT = S // P             # 16 token tiles of 128
QH = 1024               # q-chunk processed per attention unit
NQH = S // QH           # 2
SCALE = 1.0 / float(np.sqrt(D))
ESHIFT = -2.0           # exp(s/8 - 2): keeps P below fp8e4m3 max (448);
                        # cancels in the softmax normalization

DT = mybir.dt.bfloat16
NP_DT = ml_dtypes.bfloat16
F32 = mybir.dt.float32
F32R = mybir.dt.float32r
F8 = mybir.dt.float8e4
NTP = NT // 2           # kt pairs for DoubleRow mm2
VPAD = 72               # padded per-head V row (16B-aligned pair stride)

AF = mybir.ActivationFunctionType


def build_nc():
    nc = bacc.Bacc(
        "TRN2", target_bir_lowering=False, debug=False, enable_asserts=False
    )
    xT = nc.dram_tensor("xT", [S // 512, P, EC, 512], DT, kind="ExternalInput").ap()
    wq = nc.dram_tensor("wq", [P, EC, EH], DT, kind="ExternalInput").ap()
    wk = nc.dram_tensor("wk", [P, EC, EH], DT, kind="ExternalInput").ap()
    wv = nc.dram_tensor("wv", [P, EC, EH], DT, kind="ExternalInput").ap()
    wo = nc.dram_tensor("wo", [P, MC, E], DT, kind="ExternalInput").ap()
    bq = nc.dram_tensor("bq", [P, MC], F32, kind="ExternalInput").ap()
    y = nc.dram_tensor("y", [S, E], F32, kind="ExternalOutput").ap()

    with tile.TileContext(nc) as tc:
        with (
            tc.tile_pool(name="consts", bufs=1) as consts,
            tc.tile_pool(name="work", bufs=6) as work,
            tc.tile_pool(name="norm", bufs=2) as norm,
            tc.tile_pool(name="outsb", bufs=2) as outsb,
            tc.tile_pool(name="psA", bufs=2, space="PSUM") as psA,
            tc.tile_pool(name="psO", bufs=4, space="PSUM") as psO,
            tc.tile_pool(name="dram", bufs=2, space="DRAM") as dram,
        ):
            # ---- constant loads ----
            wk_sb = consts.tile([P, EC, EH], DT)
            nc.gpsimd.dma_start(out=wk_sb, in_=wk)
            wv_sb = consts.tile([P, EC, EH], DT)
            nc.gpsimd.dma_start(out=wv_sb, in_=wv)
            xT_t4 = []
            for t4 in range(S // 512):
                xt = consts.tile([P, EC, 512], DT, name=f"xT{t4}")
                nc.sync.dma_start(out=xt, in_=xT[t4])
                xT_t4.append(xt)
            wq_sb = consts.tile([P, EC, EH], DT)
            nc.scalar.dma_start(out=wq_sb, in_=wq)
            wo_sb = consts.tile([P, MC, E], DT)
            nc.scalar.dma_start(out=wo_sb, in_=wo)
            bq_sb = consts.tile([P, MC], F32)
            nc.gpsimd.dma_start(out=bq_sb, in_=bq)


            eshift_sb = consts.tile([P, 1], F32)
            nc.vector.memset(eshift_sb, ESHIFT)
            QT_sb = consts.tile([P, MC, S], DT)
            KT_sb = consts.tile([P, MC, S], DT)
            V_sb = consts.tile([P, NT, HPC, D + 1], DT)
            OT_sb = consts.tile([P, MC, S], DT)
            nc.vector.memset(V_sb[:, :, :, D : D + 1], 1.0)

            # ---- QKV projections ----
            # K first, then V, then Q -- attention on the first q-chunk can
            # start as soon as Q's first half is done. All evacuations on DVE
            # (tensor_scalar adds bq per-partition) so ACT is free for exp.
            def qk_chunk(w_sb, dst, mc, t4, is_q):
                sl = bass.ts(t4, 512)  # destination slice in QT/KT
                ps = psO.tile(
                    [P, 512], F32, tag="acc", name=f"qk{t4}{mc}{int(is_q)}"
                )
                for ec in range(EC):
                    nc.tensor.matmul(
                        ps,
                        lhsT=w_sb[:, ec, mc * P : (mc + 1) * P],
                        rhs=xT_t4[t4][:, ec, :],
                        start=(ec == 0),
                        stop=(ec == EC - 1),
                    )
                if is_q:
                    nc.vector.tensor_scalar_add(
                        out=dst[:, mc, sl], in0=ps, scalar1=bq_sb[:, mc : mc + 1]
                    )
                else:
                    nc.vector.tensor_copy(out=dst[:, mc, sl], in_=ps)

            def v_tile(t):
                ps = psO.tile([P, EH], F32, tag="acc", name=f"v{t}")
                for ec in range(EC):
                    nc.tensor.matmul(
                        ps,
                        lhsT=xT_t4[t // 4][:, ec, bass.ts(t % 4, P)],
                        rhs=wv_sb[:, ec, :],
                        start=(ec == 0),
                        stop=(ec == EC - 1),
                    )
                nc.vector.tensor_copy(
                    out=V_sb[:, t, :, 0:D],
                    in_=ps.rearrange("p (h d) -> p h d", h=HPC),
                )

            for t4 in range(2):
                for mc in range(MC):
                    qk_chunk(wq_sb, QT_sb, mc, t4, True)
            for mc in range(MC):
                qk_chunk(wk_sb, KT_sb, mc, 0, False)
            v_tile(0)
            v_tile(1)

            # ---- attention + output projection, software pipelined ----
            y_r = y.rearrange("(t p) n -> t p n", p=P)

            def evac_half(hp, iq, qs, O_pair, Ou, Rs, on_act=False):
                for i in range(2):
                    ou = work.tile(
                        [64, 512], F32, tag="ou", bufs=6,
                        name=f"ou{hp}{iq}{i}{qs}",
                    )
                    if on_act:
                        nc.scalar.copy(out=ou, in_=O_pair[i][0:D, :])
                    else:
                        nc.vector.tensor_copy(out=ou, in_=O_pair[i][0:D, :])
                    rs = norm.tile([1, 512], F32, tag="rs", bufs=5,
                                   name=f"rs{hp}{iq}{i}{qs}")
                    nc.vector.tensor_copy(out=rs, in_=O_pair[i][D : D + 1, :])
                    rc = norm.tile([1, 512], F32, tag="rc", bufs=5,
                                   name=f"rc{hp}{iq}{i}{qs}")
                    nc.vector.reciprocal_approx_fast(out=rc, in_=rs)
                    Ou.append(ou)
                    Rs.append(rc)

            def emit_pass_b(hp, O_pair, PTs, kts=None):
                for kt in kts if kts is not None else range(NT):
                    for i in range(2):
                        nc.tensor.matmul(
                            O_pair[i][:, :],
                            lhsT=V_sb[:, kt, 2 * hp + i, :],
                            rhs=PTs[kt][i][:, 512:1024],
                            start=(kt == 0),
                            stop=(kt == NT - 1),
                        )

            def att_subunit(hp, iq, sub, deferred=()):
                """Single-pass QH=512 unit: q range [iq*QH + sub*512, +512).
                Returns (O_pair, Ou, Rs) with evacuation done."""
                deferred = dict(deferred)
                q0 = iq * QH + sub * 512
                Ou, Rs = [], []
                O_pair = [
                    psO.tile([D + 1, 512], F32, tag="acc", name=f"Os{sub}a"),
                    psO.tile([D + 1, 512], F32, tag="acc", name=f"Os{sub}b"),
                ]
                for kt in range(NT):
                    if kt in deferred:
                        deferred[kt]()
                    ST_pair = [
                        psA.tile([P, 512], F32, tag="big", name=f"STs{sub}{kt}a"),
                        psA.tile([P, 512], F32, tag="big", name=f"STs{sub}{kt}b"),
                    ]
                    for i, base in ((0, 0), (1, 64)):
                        nc.tensor.matmul(
                            ST_pair[i],
                            lhsT=KT_sb[base : base + 64, hp, bass.ts(kt, P)],
                            rhs=QT_sb[base : base + 64, hp, q0 : q0 + 512],
                            start=True,
                            stop=True,
                        )
                    for i in range(2):
                        PT = work.tile(
                            [P, 512], DT, tag="pts", bufs=4,
                            name=f"PTs{sub}{kt}{i}",
                        )
                        nc.scalar.activation(
                            out=PT, in_=ST_pair[i], func=AF.Exp, scale=SCALE
                        )
                        nc.tensor.matmul(
                            O_pair[i][:, :],
                            lhsT=V_sb[:, kt, 2 * hp + i, :],
                            rhs=PT,
                            start=(kt == 0),
                            stop=(kt == NT - 1),
                        )
                for i in range(2):
                    ou = work.tile(
                        [64, 512], F32, tag="ou", bufs=6, name=f"ous{sub}{i}"
                    )
                    nc.scalar.copy(out=ou, in_=O_pair[i][0:D, :])
                    rs = norm.tile([1, 512], F32, tag="rs", bufs=5,
                                   name=f"rss{sub}{i}")
                    nc.vector.tensor_copy(out=rs, in_=O_pair[i][D : D + 1, :])
                    rc = norm.tile([1, 512], F32, tag="rc", bufs=5,
                                   name=f"rcs{sub}{i}")
                    nc.vector.reciprocal_approx_fast(out=rc, in_=rs)
                    Ou.append(ou)
                    Rs.append(rc)
                return O_pair, Ou, Rs

            def norm_subunit(hp, iq, sub, Ou, Rs):
                q0 = iq * QH + sub * 512
                rdram = dram.tile([2, 512], F32, tag="rdsub", bufs=2,
                                  name=f"rds{sub}")
                for i in range(2):
                    nc.sync.dma_start(out=rdram[i : i + 1, :], in_=Rs[i])
                bc = norm.tile([64, 2, 512], F32, tag="bcs", bufs=1,
                               name=f"bcs{sub}")
                rdram_b = bass.AP(
                    tensor=rdram.tensor,
                    offset=rdram.offset,
                    ap=[[0, 64]] + list(rdram.ap),
                )
                nc.sync.dma_start(out=bc, in_=rdram_b)
                for i in range(2):
                    nc.vector.tensor_mul(
                        out=OT_sb[64 * i : 64 * i + 64, hp, q0 : q0 + 512],
                        in0=Ou[i],
                        in1=bc[:, i, :],
                    )

            def att_unit(hp, iq, Ou, Rs, deferred=()):
                """Scores^T -> exp -> [V|1]^T @ P^T for heads (2hp, 2hp+1) on
                q-chunk iq; evacuates unnormalized O^T + row sums to SBUF.
                `deferred` maps kt -> emit-callback for pipelined fill work."""
                deferred = dict(deferred)
                PT_pairs = [None, None]
                q0 = iq * QH
                O_pair = [
                    psO.tile([D + 1, 512], F32, tag="acc", name=f"O{hp}{iq}a"),
                    psO.tile([D + 1, 512], F32, tag="acc", name=f"O{hp}{iq}b"),
                ]
                PTs = []
                for kt in range(NT):
                    if kt in deferred:
                        deferred[kt]()
                    ST_pair = [
                        psA.tile([P, QH], F32, tag="big", name=f"ST{hp}{iq}{kt}a"),
                        psA.tile([P, QH], F32, tag="big", name=f"ST{hp}{iq}{kt}b"),
                    ]
                    # scores^T: row-group packed pair (bases 0 / 64)
                    for qs in range(QH // 512):
                        for i, base in ((0, 0), (1, 64)):
                            nc.tensor.matmul(
                                ST_pair[i][:, bass.ts(qs, 512)],
                                lhsT=KT_sb[base : base + 64, hp, bass.ts(kt, P)],
                                rhs=QT_sb[
                                    base : base + 64,
                                    hp,
                                    q0 + qs * 512 : q0 + (qs + 1) * 512,
                                ],
                                start=True,
                                stop=True,
                            )
                    PT_kt = []
                    for i in range(2):
                        h_local = 2 * hp + i
                        PT = work.tile(
                            [P, QH], DT, tag="pt", bufs=31,
                            name=f"PT{hp}{iq}{kt}{i}",
                        )
                        nc.scalar.activation(
                            out=PT, in_=ST_pair[i], func=AF.Exp, scale=SCALE
                        )
                        nc.tensor.matmul(
                            O_pair[i][:, :],
                            lhsT=V_sb[:, kt, h_local, :],
                            rhs=PT[:, 0:512],
                            start=(kt == 0),
                            stop=(kt == NT - 1),
                        )
                        PT_kt.append(PT)
                    PTs.append(PT_kt)
                # evac pass-A half; pass-B + its evac + normalize run later
                # (spread into the next unit's kt loop, or inline at the end)
                evac_half(hp, iq, 0, O_pair, Ou, Rs)
                return O_pair, PTs

            def normalize(iq, hp, Ou, Rs):
                """Approx-recip rows -> DMA broadcast -> DVE renorm into OT_sb
                for one (iq, hp) unit; runs concurrently with the next unit.
                Ou/Rs hold quarters in order (qs, head): A-e, A-o, B-e, B-o."""
                q0 = iq * QH
                rdram = dram.tile([2, QH], F32, tag="rdram", bufs=4,
                                  name=f"rd{iq}{hp}")
                for u, (qs, i) in enumerate(((0, 0), (0, 1), (1, 0), (1, 1))):
                    nc.sync.dma_start(
                        out=rdram[i : i + 1, qs * 512 : qs * 512 + 512],
                        in_=Rs[u],
                    )
                bc = norm.tile([64, 2, QH], F32, tag="bc", bufs=2,
                               name=f"bc{iq}{hp}")
                rdram_b = bass.AP(
                    tensor=rdram.tensor,
                    offset=rdram.offset,
                    ap=[[0, 64]] + list(rdram.ap),
                )
                nc.sync.dma_start(out=bc, in_=rdram_b)
                for u, (qs, i) in enumerate(((0, 0), (0, 1), (1, 0), (1, 1))):
                    nc.vector.tensor_mul(
                        out=OT_sb[
                            64 * i : 64 * i + 64,
                            hp,
                            q0 + qs * 512 : q0 + qs * 512 + 512,
                        ],
                        in0=Ou[u],
                        in1=bc[:, i, qs * 512 : qs * 512 + 512],
                    )

            def y_tile(t, act_evac=False):
                    y_sb = outsb.tile([P, E], F32, tag="ysb", name=f"ysb{t}")
                    for n2 in range(E // 512):
                        psY = psO.tile(
                            [P, 512], F32, tag="acc", name=f"psY{t}{n2}"
                        )
                        for mc in range(MC):
                            nc.tensor.matmul(
                                psY,
                                lhsT=OT_sb[:, mc, bass.ts(t, P)],
                                rhs=wo_sb[:, mc, bass.ts(n2, 512)],
                                start=(mc == 0),
                                stop=(mc == MC - 1),
                            )
                        if act_evac:
                            nc.scalar.copy(
                                out=y_sb[:, bass.ts(n2, 512)], in_=psY
                            )
                        else:
                            nc.vector.tensor_copy(
                                out=y_sb[:, bass.ts(n2, 512)], in_=psY
                            )
                    nc.sync.dma_start(out=y_r[t], in_=y_sb)

            def y_proj(iq):
                for t in range(iq * (NT // NQH), (iq + 1) * (NT // NQH)):
                    y_tile(t, act_evac=(iq == NQH - 1))

            # Pipeline: Y(iq) is emitted after att(0, iq+1) so the PE has a
            # full unit of attention work queued before it reaches Y's
            # dependency on the normalize chain (engines run in order).
            def fill(emits):
                d = {}
                for kt, fn in emits:
                    d.setdefault(kt, []).append(fn)
                return {
                    kt: (lambda fns=fns: [f() for f in fns])
                    for kt, fns in d.items()
                }

            def finish_unit(pend, on_act=False, emit_b=False):
                hp_p, iq_p, O_p, PTs_p, Ou_p, Rs_p = pend
                if emit_b:
                    emit_pass_b(hp_p, O_p, PTs_p)
                evac_half(hp_p, iq_p, 1, O_p, Ou_p, Rs_p, on_act=on_act)
                normalize(iq_p, hp_p, Ou_p, Rs_p)

            def spread_pending(emits, pend):
                # previous unit's pass-B across kts 0..7 (4 MMs/kt), then its
                # second-half evacuation + normalize at kt 8
                hp_p, iq_p, O_p, PTs_p, Ou_p, Rs_p = pend
                for j in range(8):
                    kts = range(2 * j, 2 * j + 2)
                    emits.append((
                        j,
                        lambda k=kts: emit_pass_b(hp_p, O_p, PTs_p, k),
                    ))
                emits.append((8, lambda: finish_unit(pend)))

            units = [(iq, hp) for iq in range(NQH) for hp in range(MC)]
            pending = None
            for u, (iq, hp) in enumerate(units[:-1]):
                Ou, Rs = [], []
                emits = []
                if pending is not None:
                    spread_pending(emits, pending)
                    pending = None
                if hp == 0 and iq == 0:
                    # V tiles at lead-2 pacing; K chunks just before need
                    for t in range(2, NT):
                        emits.append((max(t - 2, 10) if t > 11 else t - 2,
                                      lambda t=t: v_tile(t)))
                    for t4 in range(1, 4):
                        for mc in range(MC):
                            emits.append((
                                4 * t4 - 3 + mc,
                                lambda m=mc, t=t4:
                                qk_chunk(wk_sb, KT_sb, m, t, False),
                            ))
                if hp == 1 and iq == 0:
                    for j, t4 in enumerate((2, 3)):
                        for mc in range(MC):
                            emits.append((
                                10 + 2 * j + mc,
                                lambda m=mc, t=t4:
                                qk_chunk(wq_sb, QT_sb, m, t, True),
                            ))
                if hp == 0 and iq > 0:
                    base_t = (iq - 1) * (NT // NQH)
                    for j in range(NT // NQH):
                        emits.append((
                            9 + (j * 6) // 8,
                            lambda t=base_t + j: y_tile(t),
                        ))
                O_pair, PTs = att_unit(hp, iq, Ou, Rs, fill(emits))
                pending = (hp, iq, O_pair, PTs, Ou, Rs)

            # last unit as two QH=512 single-pass sub-units; Y pipelined in
            liq, lhp = units[-1]
            emits = []
            spread_pending(emits, pending)
            _, OuA, RsA = att_subunit(lhp, liq, 0, fill(emits))
            norm_subunit(lhp, liq, 0, OuA, RsA)
            base_t = liq * (NT // NQH)
            emits = []
            for j in range(4):
                emits.append((2 + 2 * j, lambda t=base_t + j: y_tile(t)))
            _, OuB, RsB = att_subunit(lhp, liq, 1, fill(emits))
            norm_subunit(lhp, liq, 1, OuB, RsB)
            for t in range(base_t + 4, base_t + NT // NQH):
                y_tile(t, act_evac=True)

    nc.compile()
    return nc


_NC_CACHE = {}


def get_nc():
    if "nc" not in _NC_CACHE:
        _NC_CACHE["nc"] = build_nc()
    return _NC_CACHE["nc"]


def make_in_maps(x, Wq, bq, Wk, Wv, Wo):
    # all arrays pre-permuted into the exact SBUF layouts so device DMAs
    # are fully contiguous (cheap descriptor generation)
    xT_by_batch = [
        np.ascontiguousarray(
            x[b].T.astype(NP_DT).reshape(EC, P, S // 512, 512).transpose(2, 1, 0, 3)
        )
        for b in range(B)
    ]
    in_maps = []
    for c in range(NCORES):
        b, hg = divmod(c, NCORES // B)
        hs = slice(hg * EH, (hg + 1) * EH)

        def wlayout(W):
            return np.ascontiguousarray(
                W[:, hs].astype(NP_DT).reshape(EC, P, EH).transpose(1, 0, 2)
            )

        in_maps.append(
            {
                "xT": xT_by_batch[b],
                "wq": wlayout(Wq),
                "wk": wlayout(Wk),
                "wv": wlayout(Wv),
                "wo": np.ascontiguousarray(
                    Wo[hs, :].astype(NP_DT).reshape(MC, P, E).transpose(1, 0, 2)
                ),
                "bq": np.ascontiguousarray(
                    bq[hs].astype(np.float32).reshape(MC, P).T
                ),
            }
        )
    return in_maps


def gather_out(results, bv, Wo, bo):
    bias_row = (
        bv.astype(np.float64) @ Wo.astype(np.float64) + bo.astype(np.float64)
    ).astype(np.float32)
    out = np.empty((B, S, E), np.float32)
    gpb = NCORES // B
    for b in range(B):
        acc = results[gpb * b]["y"].copy()
        for i in range(1, gpb):
            acc += results[gpb * b + i]["y"]
        out[b] = acc + bias_row
    return out


def kernel(x, Wq, bq, Wk, bk, Wv, bv, Wo, bo, **_):
    x = np.asarray(x, np.float32)
    nc = get_nc()
    in_maps = make_in_maps(
        x,
        np.asarray(Wq, np.float32),
        np.asarray(bq, np.float32),
        np.asarray(Wk, np.float32),
        np.asarray(Wv, np.float32),
        np.asarray(Wo, np.float32),
    )
    res = run_bass_kernel_spmd(nc, in_maps, list(range(NCORES)))
    return gather_out(
        res.results, np.asarray(bv, np.float32), np.asarray(Wo, np.float32),
        np.asarray(bo, np.float32)
    )


# revision 40
# speedup vs baseline: 1.0079x; 1.0079x over previous
"""Multi-head self-attention (B=2, S=2048, E=1024, H=16, D=64) on 8 TRN2 cores.

Sharding: tensor-parallel over (batch, head-group): core c handles batch c//4
and heads [4*(c%4), 4*(c%4)+4). Each core computes its heads' attention output
projected through its slice of Wo; the host sums the 4 partial outputs per
batch and adds the constant bias row (bv @ Wo + bo).

Device-side math (per core, transposed formulation so no transposes needed):
  QT = Wq_c^T @ x^T + bq_c        [256, S]   (bias bk dropped: softmax-invariant)
  KT = Wk_c^T @ x^T               [256, S]
  V  = x @ Wv_c                   [S, 256]   (bias bv folded into host bias row)
  S^T tile = K @ Q^T              (PE, per 128-k-token x 1024-q tile)
  P^T = exp(S^T / 8)              (ACT, no max subtraction: scores ~ N(0,1))
  O^T aug = [V | 1]^T @ P^T       (PE, accumulated over k tiles; row 64 = sum)
  O^T = O^T aug[0:64] / row 64    (recip + PE broadcast + DVE mul)
  Y = O @ Wo_c                    [S, 1024]  fp32 partial out
"""

import numpy as np
import ml_dtypes

import concourse.bass as bass
import concourse.bacc as bacc
import concourse.tile as tile
from concourse import mybir
from concourse.bass_utils import run_bass_kernel_spmd

B, S, E = 2, 2048, 1024
H, D = 16, 64
NCORES = 8
HPC = 4                 # heads per core
EH = HPC * D            # 256: per-core head width
P = 128
EC = E // P             # 8 E-chunks of 128
MC = EH // P            # 2 Eh-chunks of 128
NT = S // P             # 16 token tiles of 128
QH = 1024               # q-chunk processed per attention unit
NQH = S // QH           # 2
SCALE = 1.0 / float(np.sqrt(D))
ESHIFT = -2.0           # exp(s/8 - 2): keeps P below fp8e4m3 max (448);
                        # cancels in the softmax normalization

DT = mybir.dt.bfloat16
NP_DT = ml_dtypes.bfloat16
F32 = mybir.dt.float32
F32R = mybir.dt.float32r
F8 = mybir.dt.float8e4
NTP = NT // 2           # kt pairs for DoubleRow mm2
VPAD = 72               # padded per-head V row (16B-aligned pair stride)

AF = mybir.ActivationFunctionType


def build_nc():
    nc = bacc.Bacc(
        "TRN2", target_bir_lowering=False, debug=False, enable_asserts=False
    )
    xT = nc.dram_tensor("xT", [S // 512, P, EC, 512], DT, kind="ExternalInput").ap()
    wq = nc.dram_tensor("wq", [P, EC, EH], DT, kind="ExternalInput").ap()
    wk = nc.dram_tensor("wk", [P, EC, EH], DT, kind="ExternalInput").ap()
    wv = nc.dram_tensor("wv", [P, EC, EH], DT, kind="ExternalInput").ap()
    wo = nc.dram_tensor("wo", [P, MC, E], DT, kind="ExternalInput").ap()
    bq = nc.dram_tensor("bq", [P, MC], F32, kind="ExternalInput").ap()
    y = nc.dram_tensor("y", [S, E], F32, kind="ExternalOutput").ap()

    with tile.TileContext(nc) as tc:
        with (
            tc.tile_pool(name="consts", bufs=1) as consts,
            tc.tile_pool(name="work", bufs=6) as work,
            tc.tile_pool(name="norm", bufs=2) as norm,
            tc.tile_pool(name="outsb", bufs=2) as outsb,
            tc.tile_pool(name="psA", bufs=2, space="PSUM") as psA,
            tc.tile_pool(name="psO", bufs=4, space="PSUM") as psO,
            tc.tile_pool(name="dram", bufs=2, space="DRAM") as dram,
        ):
            # ---- constant loads ----
            wk_sb = consts.tile([P, EC, EH], DT)
            nc.gpsimd.dma_start(out=wk_sb, in_=wk)
            wv_sb = consts.tile([P, EC, EH], DT)
            nc.gpsimd.dma_start(out=wv_sb, in_=wv)
            xT_t4 = []
            for t4 in range(S // 512):
                xt = consts.tile([P, EC, 512], DT, name=f"xT{t4}")
                nc.sync.dma_start(out=xt, in_=xT[t4])
                xT_t4.append(xt)
            wq_sb = consts.tile([P, EC, EH], DT)
            nc.scalar.dma_start(out=wq_sb, in_=wq)
            wo_sb = consts.tile([P, MC, E], DT)
            nc.scalar.dma_start(out=wo_sb, in_=wo)
            bq_sb = consts.tile([P, MC], F32)
            nc.gpsimd.dma_start(out=bq_sb, in_=bq)


            eshift_sb = consts.tile([P, 1], F32)
            nc.vector.memset(eshift_sb, ESHIFT)
            QT_sb = consts.tile([P, MC, S], DT)
            KT_sb = consts.tile([P, MC, S], DT)
            V_sb = consts.tile([P, NT, HPC, D + 1], DT)
            OT_sb = consts.tile([P, MC, S], DT)
            nc.vector.memset(V_sb[:, :, :, D : D + 1], 1.0)

            # ---- QKV projections ----
            # K first, then V, then Q -- attention on the first q-chunk can
            # start as soon as Q's first half is done. All evacuations on DVE
            # (tensor_scalar adds bq per-partition) so ACT is free for exp.
            def qk_chunk(w_sb, dst, mc, t4, is_q):
                sl = bass.ts(t4, 512)  # destination slice in QT/KT
                ps = psO.tile(
                    [P, 512], F32, tag="acc", name=f"qk{t4}{mc}{int(is_q)}"
                )
                for ec in range(EC):
                    nc.tensor.matmul(
                        ps,
                        lhsT=w_sb[:, ec, mc * P : (mc + 1) * P],
                        rhs=xT_t4[t4][:, ec, :],
                        start=(ec == 0),
                        stop=(ec == EC - 1),
                    )
                if is_q:
                    nc.vector.tensor_scalar_add(
                        out=dst[:, mc, sl], in0=ps, scalar1=bq_sb[:, mc : mc + 1]
                    )
                else:
                    nc.vector.tensor_copy(out=dst[:, mc, sl], in_=ps)

            def v_tile(t):
                ps = psO.tile([P, EH], F32, tag="acc", name=f"v{t}")
                for ec in range(EC):
                    nc.tensor.matmul(
                        ps,
                        lhsT=xT_t4[t // 4][:, ec, bass.ts(t % 4, P)],
                        rhs=wv_sb[:, ec, :],
                        start=(ec == 0),
                        stop=(ec == EC - 1),
                    )
                nc.vector.tensor_copy(
                    out=V_sb[:, t, :, 0:D],
                    in_=ps.rearrange("p (h d) -> p h d", h=HPC),
                )

            for t4 in range(2):
                for mc in range(MC):
                    qk_chunk(wq_sb, QT_sb, mc, t4, True)
            for mc in range(MC):
                qk_chunk(wk_sb, KT_sb, mc, 0, False)
            v_tile(0)
            v_tile(1)

            # ---- attention + output projection, software pipelined ----
            y_r = y.rearrange("(t p) n -> t p n", p=P)

            def evac_half(hp, iq, qs, O_pair, Ou, Rs, on_act=False):
                for i in range(2):
                    ou = work.tile(
                        [64, 512], F32, tag="ou", bufs=6,
                        name=f"ou{hp}{iq}{i}{qs}",
                    )
                    if on_act:
                        nc.scalar.copy(out=ou, in_=O_pair[i][0:D, :])
                    else:
                        nc.vector.tensor_copy(out=ou, in_=O_pair[i][0:D, :])
                    rs = norm.tile([1, 512], F32, tag="rs", bufs=5,
                                   name=f"rs{hp}{iq}{i}{qs}")
                    nc.vector.tensor_copy(out=rs, in_=O_pair[i][D : D + 1, :])
                    rc = norm.tile([1, 512], F32, tag="rc", bufs=5,
                                   name=f"rc{hp}{iq}{i}{qs}")
                    nc.vector.reciprocal_approx_fast(out=rc, in_=rs)
                    Ou.append(ou)
                    Rs.append(rc)

            def emit_pass_b(hp, O_pair, PTs, kts=None):
                for kt in kts if kts is not None else range(NT):
                    for i in range(2):
                        nc.tensor.matmul(
                            O_pair[i][:, :],
                            lhsT=V_sb[:, kt, 2 * hp + i, :],
                            rhs=PTs[kt][i][:, 512:1024],
                            start=(kt == 0),
                            stop=(kt == NT - 1),
                        )

            def att_unit(hp, iq, Ou, Rs, deferred=()):
                """Scores^T -> exp -> [V|1]^T @ P^T for heads (2hp, 2hp+1) on
                q-chunk iq; evacuates unnormalized O^T + row sums to SBUF.
                `deferred` maps kt -> emit-callback for pipelined fill work."""
                deferred = dict(deferred)
                PT_pairs = [None, None]
                q0 = iq * QH
                O_pair = [
                    psO.tile([D + 1, 512], F32, tag="acc", name=f"O{hp}{iq}a"),
                    psO.tile([D + 1, 512], F32, tag="acc", name=f"O{hp}{iq}b"),
                ]
                PTs = []
                for kt in range(NT):
                    if kt in deferred:
                        deferred[kt]()
                    ST_pair = [
                        psA.tile([P, QH], F32, tag="big", name=f"ST{hp}{iq}{kt}a"),
                        psA.tile([P, QH], F32, tag="big", name=f"ST{hp}{iq}{kt}b"),
                    ]
                    # scores^T: row-group packed pair (bases 0 / 64)
                    for qs in range(QH // 512):
                        for i, base in ((0, 0), (1, 64)):
                            nc.tensor.matmul(
                                ST_pair[i][:, bass.ts(qs, 512)],
                                lhsT=KT_sb[base : base + 64, hp, bass.ts(kt, P)],
                                rhs=QT_sb[
                                    base : base + 64,
                                    hp,
                                    q0 + qs * 512 : q0 + (qs + 1) * 512,
                                ],
                                start=True,
                                stop=True,
                            )
                    PT_kt = []
                    for i in range(2):
                        h_local = 2 * hp + i
                        PT = work.tile(
                            [P, QH], DT, tag="pt", bufs=34,
                            name=f"PT{hp}{iq}{kt}{i}",
                        )
                        nc.scalar.activation(
                            out=PT, in_=ST_pair[i], func=AF.Exp, scale=SCALE
                        )
                        nc.tensor.matmul(
                            O_pair[i][:, :],
                            lhsT=V_sb[:, kt, h_local, :],
                            rhs=PT[:, 0:512],
                            start=(kt == 0),
                            stop=(kt == NT - 1),
                        )
                        PT_kt.append(PT)
                    PTs.append(PT_kt)
                # evac pass-A half; pass-B + its evac + normalize run later
                # (spread into the next unit's kt loop, or inline at the end)
                evac_half(hp, iq, 0, O_pair, Ou, Rs)
                return O_pair, PTs

            def normalize(iq, hp, Ou, Rs):
                """Approx-recip rows -> DMA broadcast -> DVE renorm into OT_sb
                for one (iq, hp) unit; runs concurrently with the next unit.
                Ou/Rs hold quarters in order (qs, head): A-e, A-o, B-e, B-o."""
                q0 = iq * QH
                rdram = dram.tile([2, QH], F32, tag="rdram", bufs=4,
                                  name=f"rd{iq}{hp}")
                for u, (qs, i) in enumerate(((0, 0), (0, 1), (1, 0), (1, 1))):
                    nc.sync.dma_start(
                        out=rdram[i : i + 1, qs * 512 : qs * 512 + 512],
                        in_=Rs[u],
                    )
                bc = norm.tile([64, 2, QH], F32, tag="bc", bufs=2,
                               name=f"bc{iq}{hp}")
                rdram_b = bass.AP(
                    tensor=rdram.tensor,
                    offset=rdram.offset,
                    ap=[[0, 64]] + list(rdram.ap),
                )
                nc.sync.dma_start(out=bc, in_=rdram_b)
                for u, (qs, i) in enumerate(((0, 0), (0, 1), (1, 0), (1, 1))):
                    nc.vector.tensor_mul(
                        out=OT_sb[
                            64 * i : 64 * i + 64,
                            hp,
                            q0 + qs * 512 : q0 + qs * 512 + 512,
                        ],
                        in0=Ou[u],
                        in1=bc[:, i, qs * 512 : qs * 512 + 512],
                    )

            def y_tile(t, act_evac=False):
                    y_sb = outsb.tile([P, E], F32, tag="ysb", name=f"ysb{t}")
                    for n2 in range(E // 512):
                        psY = psO.tile(
                            [P, 512], F32, tag="acc", name=f"psY{t}{n2}"
                        )
                        for mc in range(MC):
                            nc.tensor.matmul(
                                psY,
                                lhsT=OT_sb[:, mc, bass.ts(t, P)],
                                rhs=wo_sb[:, mc, bass.ts(n2, 512)],
                                start=(mc == 0),
                                stop=(mc == MC - 1),
                            )
                        if act_evac:
                            nc.scalar.copy(
                                out=y_sb[:, bass.ts(n2, 512)], in_=psY
                            )
                        else:
                            nc.vector.tensor_copy(
                                out=y_sb[:, bass.ts(n2, 512)], in_=psY
                            )
                    nc.sync.dma_start(out=y_r[t], in_=y_sb)

            def y_proj(iq):
                for t in range(iq * (NT // NQH), (iq + 1) * (NT // NQH)):
                    y_tile(t, act_evac=(iq == NQH - 1))

            # Pipeline: Y(iq) is emitted after att(0, iq+1) so the PE has a
            # full unit of attention work queued before it reaches Y's
            # dependency on the normalize chain (engines run in order).
            def fill(emits):
                d = {}
                for kt, fn in emits:
                    d.setdefault(kt, []).append(fn)
                return {
                    kt: (lambda fns=fns: [f() for f in fns])
                    for kt, fns in d.items()
                }

            def finish_unit(pend, on_act=False, emit_b=False):
                hp_p, iq_p, O_p, PTs_p, Ou_p, Rs_p = pend
                if emit_b:
                    emit_pass_b(hp_p, O_p, PTs_p)
                evac_half(hp_p, iq_p, 1, O_p, Ou_p, Rs_p, on_act=on_act)
                normalize(iq_p, hp_p, Ou_p, Rs_p)

            def spread_pending(emits, pend):
                # previous unit's pass-B across kts 0..7 (4 MMs/kt), then its
                # second-half evacuation + normalize at kt 8
                hp_p, iq_p, O_p, PTs_p, Ou_p, Rs_p = pend
                for j in range(8):
                    kts = range(2 * j, 2 * j + 2)
                    emits.append((
                        j,
                        lambda k=kts: emit_pass_b(hp_p, O_p, PTs_p, k),
                    ))
                emits.append((8, lambda: finish_unit(pend)))

            units = [(iq, hp) for iq in range(NQH) for hp in range(MC)]
            pending = None
            for u, (iq, hp) in enumerate(units):
                Ou, Rs = [], []
                emits = []
                if pending is not None:
                    spread_pending(emits, pending)
                    pending = None
                if hp == 0 and iq == 0:
                    # V tiles at lead-2 pacing; K chunks just before need
                    for t in range(2, NT):
                        emits.append((max(t - 2, 10) if t > 11 else t - 2,
                                      lambda t=t: v_tile(t)))
                    for t4 in range(1, 4):
                        for mc in range(MC):
                            emits.append((
                                4 * t4 - 3 + mc,
                                lambda m=mc, t=t4:
                                qk_chunk(wk_sb, KT_sb, m, t, False),
                            ))
                if hp == 1 and iq == 0:
                    for j, t4 in enumerate((2, 3)):
                        for mc in range(MC):
                            emits.append((
                                10 + 2 * j + mc,
                                lambda m=mc, t=t4:
                                qk_chunk(wq_sb, QT_sb, m, t, True),
                            ))
                if hp == 0 and iq > 0:
                    base_t = (iq - 1) * (NT // NQH)
                    for j in range(NT // NQH):
                        emits.append((
                            9 + (j * 6) // 8,
                            lambda t=base_t + j: y_tile(t),
                        ))
                O_pair, PTs = att_unit(hp, iq, Ou, Rs, fill(emits))
                pending = (hp, iq, O_pair, PTs, Ou, Rs)
            finish_unit(pending, on_act=True, emit_b=True)
            y_proj(NQH - 1)

    nc.compile()
    return nc


_NC_CACHE = {}


def get_nc():
    if "nc" not in _NC_CACHE:
        _NC_CACHE["nc"] = build_nc()
    return _NC_CACHE["nc"]


def make_in_maps(x, Wq, bq, Wk, Wv, Wo):
    # all arrays pre-permuted into the exact SBUF layouts so device DMAs
    # are fully contiguous (cheap descriptor generation)
    xT_by_batch = [
        np.ascontiguousarray(
            x[b].T.astype(NP_DT).reshape(EC, P, S // 512, 512).transpose(2, 1, 0, 3)
        )
        for b in range(B)
    ]
    in_maps = []
    for c in range(NCORES):
        b, hg = divmod(c, NCORES // B)
        hs = slice(hg * EH, (hg + 1) * EH)

        def wlayout(W):
            return np.ascontiguousarray(
                W[:, hs].astype(NP_DT).reshape(EC, P, EH).transpose(1, 0, 2)
            )

        in_maps.append(
            {
                "xT": xT_by_batch[b],
                "wq": wlayout(Wq),
                "wk": wlayout(Wk),
                "wv": wlayout(Wv),
                "wo": np.ascontiguousarray(
                    Wo[hs, :].astype(NP_DT).reshape(MC, P, E).transpose(1, 0, 2)
                ),
                "bq": np.ascontiguousarray(
                    bq[hs].astype(np.float32).reshape(MC, P).T
                ),
            }
        )
    return in_maps


def gather_out(results, bv, Wo, bo):
    bias_row = (
        bv.astype(np.float64) @ Wo.astype(np.float64) + bo.astype(np.float64)
    ).astype(np.float32)
    out = np.empty((B, S, E), np.float32)
    gpb = NCORES // B
    for b in range(B):
        acc = results[gpb * b]["y"].copy()
        for i in range(1, gpb):
            acc += results[gpb * b + i]["y"]
        out[b] = acc + bias_row
    return out


def kernel(x, Wq, bq, Wk, bk, Wv, bv, Wo, bo, **_):
    x = np.asarray(x, np.float32)
    nc = get_nc()
    in_maps = make_in_maps(
        x,
        np.asarray(Wq, np.float32),
        np.asarray(bq, np.float32),
        np.asarray(Wk, np.float32),
        np.asarray(Wv, np.float32),
        np.asarray(Wo, np.float32),
    )
    res = run_bass_kernel_spmd(nc, in_maps, list(range(NCORES)))
    return gather_out(
        res.results, np.asarray(bv, np.float32), np.asarray(Wo, np.float32),
        np.asarray(bo, np.float32)
    )


# revision 41
# speedup vs baseline: 1.0143x; 1.0063x over previous
"""Multi-head self-attention (B=2, S=2048, E=1024, H=16, D=64) on 8 TRN2 cores.

Sharding: tensor-parallel over (batch, head-group): core c handles batch c//4
and heads [4*(c%4), 4*(c%4)+4). Each core computes its heads' attention output
projected through its slice of Wo; the host sums the 4 partial outputs per
batch and adds the constant bias row (bv @ Wo + bo).

Device-side math (per core, transposed formulation so no transposes needed):
  QT = Wq_c^T @ x^T + bq_c        [256, S]   (bias bk dropped: softmax-invariant)
  KT = Wk_c^T @ x^T               [256, S]
  V  = x @ Wv_c                   [S, 256]   (bias bv folded into host bias row)
  S^T tile = K @ Q^T              (PE, per 128-k-token x 1024-q tile)
  P^T = exp(S^T / 8)              (ACT, no max subtraction: scores ~ N(0,1))
  O^T aug = [V | 1]^T @ P^T       (PE, accumulated over k tiles; row 64 = sum)
  O^T = O^T aug[0:64] / row 64    (approx-recip + DMA-broadcast + DVE mul)
  Y = O @ Wo_c                    [S, 1024]  fp32 partial out
"""

import numpy as np
import ml_dtypes

import concourse.bass as bass
import concourse.bacc as bacc
import concourse.tile as tile
from concourse import mybir
from concourse.bass_utils import run_bass_kernel_spmd

B, S, E = 2, 2048, 1024
H, D = 16, 64
NCORES = 8
HPC = 4                 # heads per core
EH = HPC * D            # 256: per-core head width
P = 128
EC = E // P             # 8 E-chunks of 128
MC = EH // P            # 2 Eh-chunks of 128
NT = S // P             # 16 token tiles of 128
QH = 1024               # q-chunk processed per attention unit
NQH = S // QH           # 2
SCALE = 1.0 / float(np.sqrt(D))

DT = mybir.dt.bfloat16
NP_DT = ml_dtypes.bfloat16
F32 = mybir.dt.float32
F32R = mybir.dt.float32r

AF = mybir.ActivationFunctionType


def build_nc():
    nc = bacc.Bacc(
        "TRN2", target_bir_lowering=False, debug=False, enable_asserts=False
    )
    xT = nc.dram_tensor("xT", [S // 512, P, EC, 512], DT, kind="ExternalInput").ap()
    wq = nc.dram_tensor("wq", [P, EC, EH], DT, kind="ExternalInput").ap()
    wk = nc.dram_tensor("wk", [P, EC, EH], DT, kind="ExternalInput").ap()
    wv = nc.dram_tensor("wv", [P, EC, EH], DT, kind="ExternalInput").ap()
    wo = nc.dram_tensor("wo", [P, MC, E], DT, kind="ExternalInput").ap()
    bq = nc.dram_tensor("bq", [P, MC], F32, kind="ExternalInput").ap()
    y = nc.dram_tensor("y", [S, E], F32, kind="ExternalOutput").ap()

    with tile.TileContext(nc) as tc:
        with (
            tc.tile_pool(name="consts", bufs=1) as consts,
            tc.tile_pool(name="work", bufs=6) as work,
            tc.tile_pool(name="norm", bufs=2) as norm,
            tc.tile_pool(name="outsb", bufs=2) as outsb,
            tc.tile_pool(name="psA", bufs=2, space="PSUM") as psA,
            tc.tile_pool(name="psO", bufs=4, space="PSUM") as psO,
            tc.tile_pool(name="dram", bufs=2, space="DRAM") as dram,
        ):
            # ---- constant loads ----
            wk_sb = consts.tile([P, EC, EH], DT)
            nc.gpsimd.dma_start(out=wk_sb, in_=wk)
            wv_sb = consts.tile([P, EC, EH], DT)
            nc.gpsimd.dma_start(out=wv_sb, in_=wv)
            xT_t4 = []
            for t4 in range(S // 512):
                xt = consts.tile([P, EC, 512], DT, name=f"xT{t4}")
                nc.sync.dma_start(out=xt, in_=xT[t4])
                xT_t4.append(xt)
            wq_sb = consts.tile([P, EC, EH], DT)
            nc.scalar.dma_start(out=wq_sb, in_=wq)
            wo_sb = consts.tile([P, MC, E], DT)
            nc.scalar.dma_start(out=wo_sb, in_=wo)
            bq_sb = consts.tile([P, MC], F32)
            nc.gpsimd.dma_start(out=bq_sb, in_=bq)


            QT_sb = consts.tile([P, MC, S], DT)
            KT_sb = consts.tile([P, MC, S], DT)
            V_sb = consts.tile([P, NT, HPC, D + 1], DT)
            OT_sb = consts.tile([P, MC, S], DT)
            nc.vector.memset(V_sb[:, :, :, D : D + 1], 1.0)

            # ---- QKV projections ----
            # K first, then V, then Q -- attention on the first q-chunk can
            # start as soon as Q's first half is done. All evacuations on DVE
            # (tensor_scalar adds bq per-partition) so ACT is free for exp.
            def qk_chunk(w_sb, dst, mc, t4, is_q):
                sl = bass.ts(t4, 512)  # destination slice in QT/KT
                ps = psO.tile(
                    [P, 512], F32, tag="acc", name=f"qk{t4}{mc}{int(is_q)}"
                )
                for ec in range(EC):
                    nc.tensor.matmul(
                        ps,
                        lhsT=w_sb[:, ec, mc * P : (mc + 1) * P],
                        rhs=xT_t4[t4][:, ec, :],
                        start=(ec == 0),
                        stop=(ec == EC - 1),
                    )
                if is_q:
                    nc.vector.tensor_scalar_add(
                        out=dst[:, mc, sl], in0=ps, scalar1=bq_sb[:, mc : mc + 1]
                    )
                else:
                    nc.vector.tensor_copy(out=dst[:, mc, sl], in_=ps)

            def v_tile(t):
                ps = psO.tile([P, EH], F32, tag="acc", name=f"v{t}")
                for ec in range(EC):
                    nc.tensor.matmul(
                        ps,
                        lhsT=xT_t4[t // 4][:, ec, bass.ts(t % 4, P)],
                        rhs=wv_sb[:, ec, :],
                        start=(ec == 0),
                        stop=(ec == EC - 1),
                    )
                nc.vector.tensor_copy(
                    out=V_sb[:, t, :, 0:D],
                    in_=ps.rearrange("p (h d) -> p h d", h=HPC),
                )

            for t4 in range(2):
                for mc in range(MC):
                    qk_chunk(wq_sb, QT_sb, mc, t4, True)
            for mc in range(MC):
                qk_chunk(wk_sb, KT_sb, mc, 0, False)
            v_tile(0)
            v_tile(1)

            # ---- attention + output projection, software pipelined ----
            y_r = y.rearrange("(t p) n -> t p n", p=P)

            def evac_half(hp, iq, qs, O_pair, Ou, Rs, on_act=False):
                for i in range(2):
                    ou = work.tile(
                        [64, 512], F32, tag="ou", bufs=6,
                        name=f"ou{hp}{iq}{i}{qs}",
                    )
                    if on_act:
                        nc.scalar.copy(out=ou, in_=O_pair[i][0:D, :])
                    else:
                        nc.vector.tensor_copy(out=ou, in_=O_pair[i][0:D, :])
                    rs = norm.tile([1, 512], F32, tag="rs", bufs=5,
                                   name=f"rs{hp}{iq}{i}{qs}")
                    nc.vector.tensor_copy(out=rs, in_=O_pair[i][D : D + 1, :])
                    rc = norm.tile([1, 512], F32, tag="rc", bufs=5,
                                   name=f"rc{hp}{iq}{i}{qs}")
                    nc.vector.reciprocal_approx_fast(out=rc, in_=rs)
                    Ou.append(ou)
                    Rs.append(rc)

            def emit_pass_b(hp, O_pair, PTs, kts=None):
                for kt in kts if kts is not None else range(NT):
                    for i in range(2):
                        nc.tensor.matmul(
                            O_pair[i][:, :],
                            lhsT=V_sb[:, kt, 2 * hp + i, :],
                            rhs=PTs[kt][i][:, 512:1024],
                            start=(kt == 0),
                            stop=(kt == NT - 1),
                        )

            def att_unit(hp, iq, Ou, Rs, deferred=()):
                """Scores^T -> exp -> [V|1]^T @ P^T for heads (2hp, 2hp+1) on
                q-chunk iq; evacuates unnormalized O^T + row sums to SBUF.
                `deferred` maps kt -> emit-callback for pipelined fill work."""
                deferred = dict(deferred)
                PT_pairs = [None, None]
                q0 = iq * QH
                O_pair = [
                    psO.tile([D + 1, 512], F32, tag="acc", name=f"O{hp}{iq}a"),
                    psO.tile([D + 1, 512], F32, tag="acc", name=f"O{hp}{iq}b"),
                ]
                PTs = []
                for kt in range(NT):
                    if kt in deferred:
                        deferred[kt]()
                    ST_pair = [
                        psA.tile([P, QH], F32, tag="big", name=f"ST{hp}{iq}{kt}a"),
                        psA.tile([P, QH], F32, tag="big", name=f"ST{hp}{iq}{kt}b"),
                    ]
                    # scores^T: row-group packed pair (bases 0 / 64)
                    for qs in range(QH // 512):
                        for i, base in ((0, 0), (1, 64)):
                            nc.tensor.matmul(
                                ST_pair[i][:, bass.ts(qs, 512)],
                                lhsT=KT_sb[base : base + 64, hp, bass.ts(kt, P)],
                                rhs=QT_sb[
                                    base : base + 64,
                                    hp,
                                    q0 + qs * 512 : q0 + (qs + 1) * 512,
                                ],
                                start=True,
                                stop=True,
                            )
                    PT_kt = []
                    for i in range(2):
                        h_local = 2 * hp + i
                        PT = work.tile(
                            [P, QH], DT, tag="pt", bufs=34,
                            name=f"PT{hp}{iq}{kt}{i}",
                        )
                        nc.scalar.activation(
                            out=PT, in_=ST_pair[i], func=AF.Exp, scale=SCALE
                        )
                        nc.tensor.matmul(
                            O_pair[i][:, :],
                            lhsT=V_sb[:, kt, h_local, :],
                            rhs=PT[:, 0:512],
                            start=(kt == 0),
                            stop=(kt == NT - 1),
                        )
                        PT_kt.append(PT)
                    PTs.append(PT_kt)
                # evac pass-A half; pass-B + its evac + normalize run later
                # (spread into the next unit's kt loop, or inline at the end)
                evac_half(hp, iq, 0, O_pair, Ou, Rs)
                return O_pair, PTs

            def normalize(iq, hp, Ou, Rs):
                """Approx-recip rows -> DMA broadcast -> DVE renorm into OT_sb
                for one (iq, hp) unit; runs concurrently with the next unit.
                Ou/Rs hold quarters in order (qs, head): A-e, A-o, B-e, B-o."""
                q0 = iq * QH
                rdram = dram.tile([2, QH], F32, tag="rdram", bufs=4,
                                  name=f"rd{iq}{hp}")
                for u, (qs, i) in enumerate(((0, 0), (0, 1), (1, 0), (1, 1))):
                    nc.sync.dma_start(
                        out=rdram[i : i + 1, qs * 512 : qs * 512 + 512],
                        in_=Rs[u],
                    )
                bc = norm.tile([64, 2, QH], F32, tag="bc", bufs=2,
                               name=f"bc{iq}{hp}")
                rdram_b = bass.AP(
                    tensor=rdram.tensor,
                    offset=rdram.offset,
                    ap=[[0, 64]] + list(rdram.ap),
                )
                nc.sync.dma_start(out=bc, in_=rdram_b)
                for u, (qs, i) in enumerate(((0, 0), (0, 1), (1, 0), (1, 1))):
                    nc.vector.tensor_mul(
                        out=OT_sb[
                            64 * i : 64 * i + 64,
                            hp,
                            q0 + qs * 512 : q0 + qs * 512 + 512,
                        ],
                        in0=Ou[u],
                        in1=bc[:, i, qs * 512 : qs * 512 + 512],
                    )

            def y_tile(t, act_evac=False):
                    y_sb = outsb.tile([P, E], F32, tag="ysb", name=f"ysb{t}")
                    for n2 in range(E // 512):
                        psY = psO.tile(
                            [P, 512], F32, tag="acc", name=f"psY{t}{n2}"
                        )
                        for mc in range(MC):
                            nc.tensor.matmul(
                                psY,
                                lhsT=OT_sb[:, mc, bass.ts(t, P)],
                                rhs=wo_sb[:, mc, bass.ts(n2, 512)],
                                start=(mc == 0),
                                stop=(mc == MC - 1),
                            )
                        if act_evac:
                            nc.scalar.copy(
                                out=y_sb[:, bass.ts(n2, 512)], in_=psY
                            )
                        else:
                            nc.vector.tensor_copy(
                                out=y_sb[:, bass.ts(n2, 512)], in_=psY
                            )
                    nc.sync.dma_start(out=y_r[t], in_=y_sb)

            def y_proj(iq):
                for t in range(iq * (NT // NQH), (iq + 1) * (NT // NQH)):
                    y_tile(t, act_evac=(iq == NQH - 1))

            # Pipeline: Y(iq) is emitted after att(0, iq+1) so the PE has a
            # full unit of attention work queued before it reaches Y's
            # dependency on the normalize chain (engines run in order).
            def fill(emits):
                d = {}
                for kt, fn in emits:
                    d.setdefault(kt, []).append(fn)
                return {
                    kt: (lambda fns=fns: [f() for f in fns])
                    for kt, fns in d.items()
                }

            def finish_unit(pend, on_act=False, emit_b=False):
                hp_p, iq_p, O_p, PTs_p, Ou_p, Rs_p = pend
                if emit_b:
                    emit_pass_b(hp_p, O_p, PTs_p)
                evac_half(hp_p, iq_p, 1, O_p, Ou_p, Rs_p, on_act=on_act)
                normalize(iq_p, hp_p, Ou_p, Rs_p)

            def spread_pending(emits, pend):
                # previous unit's pass-B across kts 0..7 (4 MMs/kt), then its
                # second-half evacuation + normalize at kt 8
                hp_p, iq_p, O_p, PTs_p, Ou_p, Rs_p = pend
                for j in range(8):
                    kts = range(2 * j, 2 * j + 2)
                    emits.append((
                        j,
                        lambda k=kts: emit_pass_b(hp_p, O_p, PTs_p, k),
                    ))
                emits.append((8, lambda: finish_unit(pend)))

            units = [(iq, hp) for iq in range(NQH) for hp in range(MC)]
            pending = None
            for u, (iq, hp) in enumerate(units):
                Ou, Rs = [], []
                emits = []
                if pending is not None:
                    spread_pending(emits, pending)
                    pending = None
                if hp == 0 and iq == 0:
                    # V tiles at lead-2 pacing; K chunks just before need
                    for t in range(2, NT):
                        emits.append((max(t - 2, 10) if t > 11 else t - 2,
                                      lambda t=t: v_tile(t)))
                    for t4 in range(1, 4):
                        for mc in range(MC):
                            emits.append((
                                4 * t4 - 3 + mc,
                                lambda m=mc, t=t4:
                                qk_chunk(wk_sb, KT_sb, m, t, False),
                            ))
                if hp == 1 and iq == 0:
                    for j, t4 in enumerate((2, 3)):
                        for mc in range(MC):
                            emits.append((
                                10 + 2 * j + mc,
                                lambda m=mc, t=t4:
                                qk_chunk(wq_sb, QT_sb, m, t, True),
                            ))
                if hp == 0 and iq > 0:
                    base_t = (iq - 1) * (NT // NQH)
                    for j in range(NT // NQH):
                        emits.append((
                            9 + (j * 6) // 8,
                            lambda t=base_t + j: y_tile(t),
                        ))
                O_pair, PTs = att_unit(hp, iq, Ou, Rs, fill(emits))
                pending = (hp, iq, O_pair, PTs, Ou, Rs)
            finish_unit(pending, on_act=True, emit_b=True)
            y_proj(NQH - 1)

    nc.compile()
    return nc


_NC_CACHE = {}


def get_nc():
    if "nc" not in _NC_CACHE:
        _NC_CACHE["nc"] = build_nc()
    return _NC_CACHE["nc"]


def make_in_maps(x, Wq, bq, Wk, Wv, Wo):
    # all arrays pre-permuted into the exact SBUF layouts so device DMAs
    # are fully contiguous (cheap descriptor generation)
    xT_by_batch = [
        np.ascontiguousarray(
            x[b].T.astype(NP_DT).reshape(EC, P, S // 512, 512).transpose(2, 1, 0, 3)
        )
        for b in range(B)
    ]
    in_maps = []
    for c in range(NCORES):
        b, hg = divmod(c, NCORES // B)
        hs = slice(hg * EH, (hg + 1) * EH)

        def wlayout(W):
            return np.ascontiguousarray(
                W[:, hs].astype(NP_DT).reshape(EC, P, EH).transpose(1, 0, 2)
            )

        in_maps.append(
            {
                "xT": xT_by_batch[b],
                "wq": wlayout(Wq),
                "wk": wlayout(Wk),
                "wv": wlayout(Wv),
                "wo": np.ascontiguousarray(
                    Wo[hs, :].astype(NP_DT).reshape(MC, P, E).transpose(1, 0, 2)
                ),
                "bq": np.ascontiguousarray(
                    bq[hs].astype(np.float32).reshape(MC, P).T
                ),
            }
        )
    return in_maps


def gather_out(results, bv, Wo, bo):
    bias_row = (
        bv.astype(np.float64) @ Wo.astype(np.float64) + bo.astype(np.float64)
    ).astype(np.float32)
    out = np.empty((B, S, E), np.float32)
    gpb = NCORES // B
    for b in range(B):
        acc = results[gpb * b]["y"].copy()
        for i in range(1, gpb):
            acc += results[gpb * b + i]["y"]
        out[b] = acc + bias_row
    return out


def kernel(x, Wq, bq, Wk, bk, Wv, bv, Wo, bo, **_):
    x = np.asarray(x, np.float32)
    nc = get_nc()
    in_maps = make_in_maps(
        x,
        np.asarray(Wq, np.float32),
        np.asarray(bq, np.float32),
        np.asarray(Wk, np.float32),
        np.asarray(Wv, np.float32),
        np.asarray(Wo, np.float32),
    )
    res = run_bass_kernel_spmd(nc, in_maps, list(range(NCORES)))
    return gather_out(
        res.results, np.asarray(bv, np.float32), np.asarray(Wo, np.float32),
        np.asarray(bo, np.float32)
    )
